# revision 4
# baseline (speedup 1.0000x reference)
"""Trainium2 Bass kernel for nn_AttentionBlock (B=4, C=256, H=W=64, RD=32).

v2: transposed-output attention. 8 cores = (batch b, query-half h); each
core computes out.T for its 2048 queries and the host un-transposes.

Math (per core, b fixed, i in its half, j over all 4096 positions):
  q = Wq x + bq                       [32, Ni]
  k = Wk x + bk                       [32, N]
  vT_aug[j, c'] = (Wv x + bv).T, with column c'=256 equal to 1/gamma
  P[j, i] = exp(k[:,j] . q[:,i])      (unnormalized; fp32 PSUM, bf16 SBUF)
  acc[i, c'] = sum_j P[j, i] * vT_aug[j, c']   (PE, transposed output)
      rows: c' 0..255 = numerator, c' 256 = Z/gamma
  out.T[i, c] = acc[i, c] * (1 / acc[i, 256]) + x.T[i, c]

Vs v1 (baseline): the attn@v phase streams 16x32x257 = 131.6K columns
instead of 4x32x(3x512) = 196K, because the output free dim is the
258-wide channel axis instead of the 512-wide query axis. The softmax
denominator lands per-partition, so normalization is a per-partition
ACT scale (no gpsimd broadcast). P/vT are bf16 (PE rate is the same
1 cycle/row; halves SBUF traffic), energies/accumulation stay fp32.

Energy matmuls stay packed 4x into 32-row PE tiles (k/q replicated
across strips); exps are 1024 wide (2 j-blocks per ACT instruction).
PSUM: 4 banks for the 2-deep [128,1024] energy pipeline + 4 banks for
the accumulators (borrowed by the phase-1 projections).
"""

import contextlib
import os
import sys

for _p in ("/opt/trn_rl_repo", "/root/.axon_site/_ro/trn_rl_repo"):
    if os.path.isdir(_p) and _p not in sys.path:
        sys.path.insert(0, _p)

import numpy as np
import ml_dtypes

import concourse.mybir as mybir
import concourse.tile as tile
from concourse import bacc
from concourse.bass_utils import run_bass_kernel_spmd

B, C, H, W = 4, 256, 64, 64
N = H * W            # 4096 positions
RD = C // 8          # 32 reduced dim
NCORES = 8
NI = N // 2          # 2048 queries per core
GW = 512             # i-group width (PSUM bank = 512 fp32)
G = NI // GW         # 4 i-groups
JB = N // 128        # 32 j-blocks
CA = C + 1           # 257: vT columns (256 ch + Z col at index C)

f32 = mybir.dt.float32
f32r = mybir.dt.float32r
bf16 = mybir.dt.bfloat16
Exp = mybir.ActivationFunctionType.Exp
Ident = mybir.ActivationFunctionType.Identity


def build_nc(n_iter: int = 1, pp_bufs: int = 8, prime: int = 4,
             exp_w: int = 2, p_dt=bf16, cad_n: int = 4, cad_ph: int = 1,
             cad_k: int = 2, skip_p1: bool = False, skip_energy: bool = False,
             skip_exp: bool = False, skip_attn: bool = False,
             skip_fin: bool = False, qk_dt=bf16, evac_split: bool = True,
             fin_split: bool = True, outp_bufs: int = 8,
             vt_rot6: bool = False):
    nc = bacc.Bacc()

    xr = nc.dram_tensor("xr", [C, N], qk_dt, kind="ExternalInput")
    xth = nc.dram_tensor("xth", [NI, C], f32, kind="ExternalInput")
    wqt = nc.dram_tensor("wqt", [C, RD], qk_dt, kind="ExternalInput")
    wkt = nc.dram_tensor("wkt", [C, RD], qk_dt, kind="ExternalInput")
    wvt = nc.dram_tensor("wvt", [C, CA], qk_dt, kind="ExternalInput")
    bq_t = nc.dram_tensor("bq", [RD, 1], f32, kind="ExternalInput")
    bk_t = nc.dram_tensor("bk", [RD, 1], f32, kind="ExternalInput")
    bvz_t = nc.dram_tensor("bvz", [1, CA], f32, kind="ExternalInput")
    one_t = nc.dram_tensor("one_r", [1, 128], f32, kind="ExternalInput")
    out_t = nc.dram_tensor("out", [NI, C], f32, kind="ExternalOutput")

    with tile.TileContext(nc) as tc:
        with tc.tile_pool(name="const", bufs=1) as cp, \
             tc.tile_pool(name="vtp", bufs=1) as vtp, \
             tc.tile_pool(name="qk", bufs=1) as qkp, \
             tc.tile_pool(name="pp", bufs=pp_bufs) as pp, \
             tc.tile_pool(name="fin", bufs=2) as fp, \
             tc.tile_pool(name="outp", bufs=outp_bufs) as op_, \
             tc.tile_pool(name="ps_e", bufs=2, space="PSUM") as ps_e, \
             tc.tile_pool(name="ps_a", bufs=1, space="PSUM") as ps_a:

            # ---- constant loads -------------------------------------------
            xr_sb = [cp.tile([128, N], qk_dt, tag=f"xr{m}", name=f"xr{m}")
                     for m in range(2)]
            for m in range(2):
                nc.sync.dma_start(out=xr_sb[m],
                                  in_=xr[m * 128:(m + 1) * 128, :])
            # x.T chunks for the residual: chunk ic at columns ic*C
            xth_sb = cp.tile([128, (NI // 128) * C], f32, tag="xth",
                             name="xth_sb")
            for ic in range(NI // 128):
                nc.sync.dma_start(
                    out=xth_sb[:, ic * C:(ic + 1) * C],
                    in_=xth[ic * 128:(ic + 1) * 128, :])
            wqt_sb = [cp.tile([128, RD], qk_dt, tag=f"wqt{m}", name=f"wqt{m}")
                      for m in range(2)]
            wkt_sb = [cp.tile([128, RD], qk_dt, tag=f"wkt{m}", name=f"wkt{m}")
                      for m in range(2)]
            wvt_sb = [cp.tile([128, CA], qk_dt, tag=f"wvt{m}", name=f"wvt{m}")
                      for m in range(2)]
            for m in range(2):
                ms = slice(m * 128, (m + 1) * 128)
                nc.sync.dma_start(out=wqt_sb[m], in_=wqt[ms, :])
                nc.sync.dma_start(out=wkt_sb[m], in_=wkt[ms, :])
                nc.sync.dma_start(out=wvt_sb[m], in_=wvt[ms, :])
            bq_sb = cp.tile([RD, 1], f32, tag="bq", name="bq_sb")
            nc.sync.dma_start(out=bq_sb, in_=bq_t[:])
            bk_sb = cp.tile([RD, 1], f32, tag="bk", name="bk_sb")
            nc.sync.dma_start(out=bk_sb, in_=bk_t[:])
            bvz_sb = cp.tile([1, CA], f32, tag="bvz", name="bvz_sb")
            nc.sync.dma_start(out=bvz_sb, in_=bvz_t[:])
            one_sb = cp.tile([1, 128], f32, tag="one", name="one_sb")
            nc.sync.dma_start(out=one_sb, in_=one_t[:])

            # [bv, 1/gamma, 0] broadcast to all 128 partitions (plants the
            # Z column that folds the gamma multiply into the reciprocal)
            pbv = ps_a.tile([128, CA], f32, tag="a0", name="pbv")
            nc.tensor.matmul(pbv, one_sb, bvz_sb, start=True, stop=True)
            bvbc_sb = cp.tile([128, CA], f32, tag="bvbc", name="bvbc_sb")
            nc.vector.tensor_copy(bvbc_sb, pbv)

            # persistent activation tiles; k/q replicated across the four
            # 32-partition strips for packed energy matmuls
            vt = [vtp.tile([128, CA], p_dt, tag=f"vt{jb}", name=f"vt{jb}")
                  for jb in range(JB)]
            for jb in range(JB):
                nc.vector.tensor_copy(vt[jb][:, C:CA], bvbc_sb[:, C:CA])
            q4 = qkp.tile([128, NI], qk_dt, tag="q", name="q4")
            k4 = qkp.tile([128, N], qk_dt, tag="k", name="k4")

            if skip_p1:
                nc.sync.dma_start(out=q4, in_=xr[0:128, 0:NI])
                nc.sync.dma_start(out=k4, in_=xr[0:128, :])
                for jb in range(JB):
                    nc.vector.tensor_copy(vt[jb][:, 0:C], bvbc_sb[:, 0:C])

            # this core's query half: column offset into xr (set per-core
            # via the input map by passing xr pre-rolled; see make_in_maps)
            loop_cm = (tc.For_i(0, n_iter, 1) if n_iter > 1
                       else contextlib.nullcontext())
            with loop_cm:
                P1 = not skip_p1
                # ---- phase 1: projections ---------------------------------
                # q projection into strip 0 (queries are xr columns 0:NI
                # after the host roll) with per-partition bias via ACT
                for g in range(G if P1 else 0):
                    gs = slice(g * GW, (g + 1) * GW)
                    pq = ps_a.tile([RD, GW], f32, tag=f"a{g % 4}", name="pq")
                    nc.tensor.matmul(pq, wqt_sb[0], xr_sb[0][:, gs],
                                     start=True, stop=False)
                    nc.tensor.matmul(pq, wqt_sb[1], xr_sb[1][:, gs],
                                     start=False, stop=True)
                    nc.scalar.activation(q4[0:RD, gs], pq, Ident, bias=bq_sb)

                # k projection into strip 0
                for g in range(N // GW if P1 else 0):
                    gs = slice(g * GW, (g + 1) * GW)
                    pk = ps_a.tile([RD, GW], f32, tag=f"a{g % 4}", name="pk")
                    nc.tensor.matmul(pk, wkt_sb[0], xr_sb[0][:, gs],
                                     start=True, stop=False)
                    nc.tensor.matmul(pk, wkt_sb[1], xr_sb[1][:, gs],
                                     start=False, stop=True)
                    if evac_split:
                        nc.vector.tensor_scalar_add(k4[0:RD, gs], pk, bk_sb)
                    else:
                        nc.scalar.activation(k4[0:RD, gs], pk, Ident,
                                             bias=bk_sb)

                # replicate q/k to strips 1-3; these DMAs hide under the vT
                # matmuls below
                for t in range(1, 4 if P1 else 1):
                    ts_ = slice(32 * t, 32 * (t + 1))
                    nc.sync.dma_start(out=q4[ts_, :], in_=q4[0:RD, :])
                    nc.sync.dma_start(out=k4[ts_, :], in_=k4[0:RD, :])

                # vT_aug j-blocks: x.T @ WvT (+ broadcast [bv,1/gamma,0])
                for jb in range(JB if P1 else 0):
                    js = slice(jb * 128, (jb + 1) * 128)
                    if vt_rot6 and jb % 6 >= 4:
                        pv = ps_e.tile([128, CA], f32, tag="pe", name="pv")
                    else:
                        pv = ps_a.tile([128, CA], f32, tag=f"a{jb % 4}",
                                       name="pv")
                    nc.tensor.matmul(pv, xr_sb[0][:, js], wvt_sb[0],
                                     start=True, stop=False)
                    nc.tensor.matmul(pv, xr_sb[1][:, js], wvt_sb[1],
                                     start=False, stop=True)
                    # 13/19 DVE/ACT split balances phase-1 evacuation
                    # (DVE also carries the 8 k-bias ops, ACT the 4 q-bias)
                    if evac_split and (jb * 13) % 32 >= 13:
                        nc.scalar.activation(vt[jb][:, 0:C], pv[:, 0:C],
                                             Ident)
                    else:
                        nc.vector.tensor_copy(vt[jb][:, 0:C], pv[:, 0:C])

                # ---- energy + exp pipeline --------------------------------
                # pair = 2 packed energy matmuls (strips 2m, 2m+1) into one
                # 2-bank PSUM tile, drained by a single 1024-wide exp
                EW = exp_w * GW
                eq = [(g, jp) for g in range(G) for jp in range(JB // exp_w)]
                p_tiles = {}
                next_e = 0

                def emit_energy_pair():
                    nonlocal next_e
                    if next_e >= len(eq):
                        return
                    g, jp = eq[next_e]
                    next_e += 1
                    gs = slice(g * GW, (g + 1) * GW)
                    pe2 = ps_e.tile([128, EW], f32, tag="pe", name="pe2")
                    ew = 8 if skip_energy else GW
                    for h in range(exp_w):
                        jc = jp * exp_w + h
                        t = jc % 4
                        js = slice(jc * 128, (jc + 1) * 128)
                        ts_ = slice(32 * t, 32 * (t + 1))
                        nc.tensor.matmul(
                            pe2[:, h * GW:h * GW + ew],
                            k4[ts_, js], q4[ts_, gs][:, 0:ew],
                            start=True, stop=True,
                            tile_position=(32 * t, 0))
                    pt2 = pp.tile([128, EW], p_dt, tag="P", name="pt2")
                    if skip_exp:
                        nc.scalar.activation(pt2[:, 0:8], pe2[:, 0:8], Exp)
                    else:
                        nc.scalar.activation(pt2, pe2, Exp)
                    p_tiles[(g, jp)] = pt2

                for _ in range(prime):
                    emit_energy_pair()

                # ---- phase 2: attention (transposed output) ---------------
                for g in range(G):
                    ac = [ps_a.tile([128, CA], f32, tag=f"a{t}",
                                    name=f"ac{t}") for t in range(4)]
                    for jc in range(JB):
                        jp, h = divmod(jc, exp_w)
                        pt2 = p_tiles[(g, jp)]
                        first, last = jc == 0, jc == JB - 1
                        for t in range(0 if skip_attn else 4):
                            lo = h * GW + t * 128
                            nc.tensor.matmul(ac[t], pt2[:, lo:lo + 128],
                                             vt[jc], start=first, stop=last)
                        if h == exp_w - 1:
                            p_tiles.pop((g, jp))
                        if jc % cad_n == cad_ph:
                            for _ in range(cad_k):
                                emit_energy_pair()

                    # finalize: per-partition gamma/Z scale + residual
                    for t in range(0 if skip_fin else 4):
                        ic = g * 4 + t
                        zr = fp.tile([128, 1], f32, tag="zr", name="zr")
                        nc.vector.reciprocal(zr, ac[t][:, 256:257])
                        ot = op_.tile([128, C], f32, tag="ot", name="ot")
                        if fin_split and t % 2 == 1:
                            nc.scalar.activation(ot, ac[t][:, 0:C], Ident,
                                                 scale=zr)
                            nc.gpsimd.tensor_add(
                                ot, ot, xth_sb[:, ic * C:(ic + 1) * C])
                        else:
                            nc.vector.scalar_tensor_tensor(
                                ot, ac[t][:, 0:C], zr,
                                xth_sb[:, ic * C:(ic + 1) * C],
                                mybir.AluOpType.mult, mybir.AluOpType.add)
                        nc.sync.dma_start(
                            out=out_t[ic * 128:(ic + 1) * 128, :], in_=ot)
    nc.finalize()
    return nc


_CACHE = {}


def _get_nc(n_iter: int = 1):
    if n_iter not in _CACHE:
        _CACHE[n_iter] = build_nc(n_iter)
    return _CACHE[n_iter]


def make_in_maps(x, Wq, bq, Wk, bk, Wv, bv, gamma):
    x = np.asarray(x, dtype=np.float32)
    Wq = np.asarray(Wq, dtype=np.float32)
    bq = np.asarray(bq, dtype=np.float32)
    Wk = np.asarray(Wk, dtype=np.float32)
    bk = np.asarray(bk, dtype=np.float32)
    Wv = np.asarray(Wv, dtype=np.float32)
    bv = np.asarray(bv, dtype=np.float32)
    gamma = np.asarray(gamma, dtype=np.float32).reshape(())

    bf = ml_dtypes.bfloat16
    wqt = np.ascontiguousarray(Wq.T).astype(bf)       # [C, RD]
    wkt = np.ascontiguousarray(Wk.T).astype(bf)       # [C, RD]
    wvt = np.zeros((C, CA), dtype=np.float32)         # [Wv.T | 0 | 0]
    wvt[:, :C] = Wv.T
    wvt = wvt.astype(bf)
    bvz = np.zeros((1, CA), dtype=np.float32)         # [0, 1/gamma, 0]
    with np.errstate(divide="ignore"):
        bvz[0, C] = np.float32(1.0) / gamma           # inf if gamma==0 ->
    one_r = np.ones((1, 128), dtype=np.float32)       # recip(inf)=0 -> out=x
    bq2 = bq.reshape(RD, 1)
    bk2 = bk.reshape(RD, 1)

    in_maps = []
    for c in range(NCORES):
        b, half = divmod(c, 2)
        xb = x[b].reshape(C, N)
        # roll so this core's query half sits at columns 0:NI (the kernel
        # reads queries from xr[:, 0:NI]); k/v use all columns so the roll
        # only permutes j, and the ones-column Z is permutation-invariant
        xbr = np.ascontiguousarray(np.roll(xb, -half * NI, axis=1)).astype(bf)
        xthh = np.ascontiguousarray(
            xb[:, half * NI:(half + 1) * NI].T
            + np.float32(gamma) * bv[None, :])
        in_maps.append({
            "xr": xbr, "xth": xthh,
            "wqt": wqt, "wkt": wkt, "wvt": wvt,
            "bq": bq2, "bk": bk2, "bvz": bvz, "one_r": one_r,
        })
    return in_maps


def assemble(results):
    out = np.empty((B, C, N), dtype=np.float32)
    for c in range(NCORES):
        b, half = divmod(c, 2)
        out[b][:, half * NI:(half + 1) * NI] = results[c]["out"].T
    return out.reshape(B, C, H, W)


def kernel(x, Wq, bq, Wk, bk, Wv, bv, gamma):
    nc = _get_nc(1)
    in_maps = make_in_maps(x, Wq, bq, Wk, bk, Wv, bv, gamma)
    res = run_bass_kernel_spmd(nc, in_maps, list(range(NCORES)))
    return assemble(res.results)
